# revision 23
# baseline (speedup 1.0000x reference)
"""Label-wise attention (CAML-style) on 8 TRN2 NeuronCores.

scores = U @ x^T        [B, L, S]
alpha  = softmax(scores, axis=S)
out    = alpha @ x      [B, L, D]
returns (out, alpha)

Sharding: batch B=8 across the 8 cores (1 batch each, full U per core).
No collectives needed.

Per-core kernel (bf16 matmuls, fp32 softmax/outputs):
  - x_b resident in SBUF in natural [s,d] layout (for alpha@x) and
    transposed [d,s] layout (for U@x^T), both bf16; transposed copies are
    made with batched DMA-xbar transposes (one instruction produces all
    128-column chunks: out[p, c, j] = in[j, c*128 + p]).
  - U streamed per 128-label tile, transposed on load the same way.
  - Per l-tile: PE matmul -> scores PSUM; ScalarE Exp (+accum_out row
    sums) -> bf16 exp tile; DVE scales to fp32 alpha -> HBM via SWDGE;
    SP issues one batched DMA-xbar transpose per scores quarter; PE
    matmul2 accumulates over 32 s-chunks; ScalarE copy-scale by
    1/rowsum -> fp32 out -> HBM.  Software-pipelined (matmul2 of tile
    i-1 is emitted after matmul1 of tile i) so PE never stalls on the
    exp/transpose chain.
"""

import sys
from contextlib import ExitStack

for _p in ("/opt/trn_rl_repo", "/opt/pypackages"):
    if _p not in sys.path:
        sys.path.insert(0, _p)

import numpy as np
import ml_dtypes

import concourse.bass as bass
import concourse.mybir as mybir
import concourse.tile as tile
from concourse import bacc

B, S, D, L = 8, 4096, 512, 8921
LP = 8960  # L padded to 70 * 128
BF16 = mybir.dt.bfloat16
F32 = mybir.dt.float32


def build_nc(s=S, n_lt=LP // 128, repeat=1):
    d = D
    n_dc = d // 128   # d-chunks of 128 (contraction for matmul1)
    n_sc = s // 128   # s-chunks of 128 (contraction for matmul2)
    qw = 1024 if s % 1024 == 0 else 512  # matmul1 PSUM chunk width
    n_q = s // qw
    lp = n_lt * 128

    nc = bacc.Bacc()
    x_d = nc.declare_dram_parameter("x", [s, d], BF16, isOutput=False)
    u_d = nc.declare_dram_parameter("u", [lp, d], BF16, isOutput=False)
    out_d = nc.declare_dram_parameter("out", [lp, d], F32, isOutput=True)
    alpha_d = nc.declare_dram_parameter("alpha", [lp, s], F32, isOutput=True)

    with tile.TileContext(nc) as tc, ExitStack() as ctx:
        const = ctx.enter_context(tc.tile_pool(name="const", bufs=1))
        utp = ctx.enter_context(tc.tile_pool(name="ut", bufs=3))
        expp = ctx.enter_context(tc.tile_pool(name="exp", bufs=3))
        expTp = ctx.enter_context(tc.tile_pool(name="expT", bufs=4))
        alphap = ctx.enter_context(tc.tile_pool(name="alpha", bufs=3))
        outp = ctx.enter_context(tc.tile_pool(name="out", bufs=3))
        statp = ctx.enter_context(tc.tile_pool(name="stat", bufs=4))
        ps1 = ctx.enter_context(tc.tile_pool(name="ps1", bufs=3, space="PSUM"))
        ps2 = ctx.enter_context(tc.tile_pool(name="ps2", bufs=2, space="PSUM"))

        # x transposed (one batched xpose): xT[p, dc, j] = x[j, dc*128 + p]
        xT = const.tile([128, n_dc, s], BF16)
        nc.sync.dma_start(out=xT, in_=x_d[:, :], transpose=True)
        # x natural: x_nat[p, c, :] = x[c*128 + p, :]
        x_nat = const.tile([128, n_sc, d], BF16)
        x_nat_cp = nc.gpsimd.dma_start(
            out=x_nat, in_=x_d[:].rearrange("(c p) d -> p c d", p=128)
        )

        pools = (utp, expp, expTp, alphap, outp, statp, ps1, ps2)
        carry = [x_nat_cp]
        for _ in range(repeat):
            carry = _body(nc, n_lt, n_dc, n_sc, n_q, qw, s, d,
                          u_d, out_d, alpha_d, x_nat, xT, pools,
                          startup_copies=carry)
    nc.compile()
    return nc


def _body(nc, n_lt, n_dc, n_sc, n_q, qw, s, d,
          u_d, out_d, alpha_d, x_nat, xT, pools, lag=2,
          startup_copies=None):
    from concourse.tile import add_dep_helper
    (utp, expp, expTp, alphap, outp, statp, ps1, ps2) = pools
    pipe = {}
    # SWDGE copies emitted since the last pin point.  Each is ordered after
    # the NEXT tile's final xbar transpose (sync=False scheduler hint) so the
    # xbar-mode serialization never leaves a late copy wedged between a
    # tile's transposes, which would stall them (and then the PE).
    pending_copies = list(startup_copies or [])
    for i in range(n_lt + lag):
        if i < n_lt:
            # UT[p, dc, j] = u[i*128 + j, dc*128 + p]  (one batched xpose)
            ut = utp.tile([128, n_dc, 128], BF16)
            nc.sync.dma_start(
                out=ut, in_=u_d[i * 128 : (i + 1) * 128, :], transpose=True
            )
            exp_t = expp.tile([128, s], BF16)
            expT_t = expTp.tile([128, n_sc, 128], BF16)
            psums = statp.tile([128, n_q], F32, tag="psums")
            nq_c = qw // 128  # xpose chunks per quarter
            for q in range(n_q):
                ps = ps1.tile([128, qw], F32)
                for h in range(qw // 512):
                    for dc in range(n_dc):
                        nc.tensor.matmul(
                            ps[:, h * 512 : (h + 1) * 512],
                            lhsT=ut[:, dc, :],
                            rhs=xT[:, dc, q * qw + h * 512 : q * qw + (h + 1) * 512],
                            start=(dc == 0),
                            stop=(dc == n_dc - 1),
                        )
                nc.scalar.activation(
                    out=exp_t[:, q * qw : (q + 1) * qw],
                    in_=ps[:],
                    func=mybir.ActivationFunctionType.Exp,
                    accum_out=psums[:, q : q + 1],
                )
                # expT[p, qc, j] = exp_t[j, qc*128 + p] for this quarter
                xp = nc.sync.dma_start(
                    out=expT_t[:, q * nq_c : (q + 1) * nq_c, :],
                    in_=exp_t[:, q * qw : (q + 1) * qw],
                    transpose=True,
                )
            for cp in pending_copies:
                add_dep_helper(cp.ins, xp.ins, sync=True,
                               reason="copy after tile's transposes")
            pending_copies = []
            ssum = statp.tile([128, 1], F32, tag="ssum")
            recip = statp.tile([128, 1], F32, tag="recip")
            nc.vector.reduce_sum(out=ssum, in_=psums, axis=mybir.AxisListType.X)
            nc.vector.reciprocal(recip, ssum)
            alpha_t = alphap.tile([128, s], F32)
            nc.vector.tensor_scalar_mul(alpha_t, exp_t, recip)
            pending_copies.append(nc.gpsimd.dma_start(
                out=alpha_d[i * 128 : (i + 1) * 128, :], in_=alpha_t
            ))
            pipe[i] = (expT_t, recip)
        if i >= lag:
            expT_p, recip_p = pipe.pop(i - lag)
            po = ps2.tile([128, d], F32)
            for sc in range(n_sc):
                nc.tensor.matmul(
                    po,
                    lhsT=expT_p[:, sc, :],
                    rhs=x_nat[:, sc, :],
                    start=(sc == 0),
                    stop=(sc == n_sc - 1),
                )
            out_t = outp.tile([128, d], F32)
            nc.scalar.mul(out_t, po, recip_p)
            pending_copies.append(nc.gpsimd.dma_start(
                out=out_d[(i - lag) * 128 : (i - lag + 1) * 128, :], in_=out_t
            ))
    return pending_copies


def kernel(x: np.ndarray, U: np.ndarray):
    from concourse.bass_utils import run_bass_kernel_spmd

    xb = np.asarray(x).astype(ml_dtypes.bfloat16)
    up = np.zeros((LP, D), dtype=ml_dtypes.bfloat16)
    up[:L] = np.asarray(U).astype(ml_dtypes.bfloat16)

    nc = build_nc()
    in_maps = [{"x": np.ascontiguousarray(xb[i]), "u": up} for i in range(B)]
    res = run_bass_kernel_spmd(nc, in_maps, list(range(B))).results

    out = np.empty((B, L, D), dtype=np.float32)
    alpha = np.empty((B, L, S), dtype=np.float32)
    for i in range(B):
        out[i] = res[i]["out"][:L]
        alpha[i] = res[i]["alpha"][:L]
    return out, alpha


# revision 24
# speedup vs baseline: 2.0311x; 2.0311x over previous
"""Label-wise attention (CAML-style) on 8 TRN2 NeuronCores.

scores = U @ x^T        [B, L, S]
alpha  = softmax(scores, axis=S)
out    = alpha @ x      [B, L, D]
returns (out, alpha)

Sharding: batch B=8 across the 8 cores (1 batch each, full U per core).
No collectives needed.

Per-core kernel (bf16 matmuls, fp32 softmax/outputs):
  - x_b resident in SBUF in natural [s,d] layout (for alpha@x) and
    transposed [d,s] layout (for U@x^T), both bf16; transposed copies are
    made with batched DMA-xbar transposes (one instruction produces all
    128-column chunks: out[p, c, j] = in[j, c*128 + p]).
  - U streamed per 128-label tile, transposed on load the same way.
  - Per l-tile: PE matmul -> scores PSUM; ScalarE Exp (+accum_out row
    sums) -> bf16 exp tile; DVE scales to fp32 alpha -> HBM via SWDGE;
    SP issues one batched DMA-xbar transpose per scores quarter; PE
    matmul2 accumulates over 32 s-chunks; ScalarE copy-scale by
    1/rowsum -> fp32 out -> HBM.  Software-pipelined (matmul2 of tile
    i-1 is emitted after matmul1 of tile i) so PE never stalls on the
    exp/transpose chain.
"""

import sys
from contextlib import ExitStack

for _p in ("/opt/trn_rl_repo", "/opt/pypackages"):
    if _p not in sys.path:
        sys.path.insert(0, _p)

import numpy as np
import ml_dtypes

import concourse.bass as bass
import concourse.mybir as mybir
import concourse.tile as tile
from concourse import bacc

B, S, D, L = 8, 4096, 512, 8921
LP = 8960  # L padded to 70 * 128
BF16 = mybir.dt.bfloat16
F32 = mybir.dt.float32


def build_nc(s=S, n_lt=LP // 128, repeat=1):
    d = D
    n_dc = d // 128   # d-chunks of 128 (contraction for matmul1)
    n_sc = s // 128   # s-chunks of 128 (contraction for matmul2)
    qw = 1024 if s % 1024 == 0 else 512  # matmul1 PSUM chunk width
    n_q = s // qw
    lp = n_lt * 128

    nc = bacc.Bacc()
    x_d = nc.declare_dram_parameter("x", [s, d], BF16, isOutput=False)
    u_d = nc.declare_dram_parameter("u", [lp, d], BF16, isOutput=False)
    out_d = nc.declare_dram_parameter("out", [lp, d], F32, isOutput=True)
    alpha_d = nc.declare_dram_parameter("alpha", [lp, s], F32, isOutput=True)

    with tile.TileContext(nc) as tc, ExitStack() as ctx:
        const = ctx.enter_context(tc.tile_pool(name="const", bufs=1))
        utp = ctx.enter_context(tc.tile_pool(name="ut", bufs=3))
        expp = ctx.enter_context(tc.tile_pool(name="exp", bufs=3))
        expTp = ctx.enter_context(tc.tile_pool(name="expT", bufs=4))
        alphap = ctx.enter_context(tc.tile_pool(name="alpha", bufs=3))
        outp = ctx.enter_context(tc.tile_pool(name="out", bufs=3))
        statp = ctx.enter_context(tc.tile_pool(name="stat", bufs=4))
        ps1 = ctx.enter_context(tc.tile_pool(name="ps1", bufs=3, space="PSUM"))
        ps2 = ctx.enter_context(tc.tile_pool(name="ps2", bufs=2, space="PSUM"))

        # x transposed (one batched xpose): xT[p, dc, j] = x[j, dc*128 + p]
        xT = const.tile([128, n_dc, s], BF16)
        nc.sync.dma_start(out=xT, in_=x_d[:, :], transpose=True)
        # x natural: x_nat[p, c, :] = x[c*128 + p, :]
        x_nat = const.tile([128, n_sc, d], BF16)
        x_nat_cp = nc.gpsimd.dma_start(
            out=x_nat, in_=x_d[:].rearrange("(c p) d -> p c d", p=128)
        )

        pools = (utp, expp, expTp, alphap, outp, statp, ps1, ps2)
        carry = [x_nat_cp]
        for _ in range(repeat):
            carry = _body(nc, n_lt, n_dc, n_sc, n_q, qw, s, d,
                          u_d, out_d, alpha_d, x_nat, xT, pools,
                          startup_copies=carry)
    nc.compile()
    return nc


def _body(nc, n_lt, n_dc, n_sc, n_q, qw, s, d,
          u_d, out_d, alpha_d, x_nat, xT, pools, lag=2,
          startup_copies=None):
    import os
    pin = os.environ.get("KPIN", "1") == "1"
    from concourse.tile import add_dep_helper
    (utp, expp, expTp, alphap, outp, statp, ps1, ps2) = pools
    pipe = {}
    # SWDGE copies emitted since the last pin point.  Each is ordered after
    # the NEXT tile's final xbar transpose (sync=False scheduler hint) so the
    # xbar-mode serialization never leaves a late copy wedged between a
    # tile's transposes, which would stall them (and then the PE).
    pending_copies = list(startup_copies or [])
    for i in range(n_lt + lag):
        if i < n_lt:
            # UT[p, dc, j] = u[i*128 + j, dc*128 + p]  (one batched xpose)
            ut = utp.tile([128, n_dc, 128], BF16)
            nc.sync.dma_start(
                out=ut, in_=u_d[i * 128 : (i + 1) * 128, :], transpose=True
            )
            exp_t = expp.tile([128, s], BF16)
            expT_t = expTp.tile([128, n_sc, 128], BF16)
            psums = statp.tile([128, n_q], F32, tag="psums")
            nq_c = qw // 128  # xpose chunks per quarter
            for q in range(n_q):
                ps = ps1.tile([128, qw], F32)
                for h in range(qw // 512):
                    for dc in range(n_dc):
                        nc.tensor.matmul(
                            ps[:, h * 512 : (h + 1) * 512],
                            lhsT=ut[:, dc, :],
                            rhs=xT[:, dc, q * qw + h * 512 : q * qw + (h + 1) * 512],
                            start=(dc == 0),
                            stop=(dc == n_dc - 1),
                        )
                nc.scalar.activation(
                    out=exp_t[:, q * qw : (q + 1) * qw],
                    in_=ps[:],
                    func=mybir.ActivationFunctionType.Exp,
                    accum_out=psums[:, q : q + 1],
                )
                # expT[p, qc, j] = exp_t[j, qc*128 + p] for this quarter
                xp = nc.sync.dma_start(
                    out=expT_t[:, q * nq_c : (q + 1) * nq_c, :],
                    in_=exp_t[:, q * qw : (q + 1) * qw],
                    transpose=True,
                )
            if pin:
                for cp in pending_copies:
                    add_dep_helper(cp.ins, xp.ins, sync=True,
                                   reason="copy after tile's transposes")
            pending_copies = []
            ssum = statp.tile([128, 1], F32, tag="ssum")
            recip = statp.tile([128, 1], F32, tag="recip")
            nc.vector.reduce_sum(out=ssum, in_=psums, axis=mybir.AxisListType.X)
            nc.vector.reciprocal(recip, ssum)
            alpha_t = alphap.tile([128, s], F32)
            nc.vector.tensor_scalar_mul(alpha_t, exp_t, recip)
            pending_copies.append(nc.gpsimd.dma_start(
                out=alpha_d[i * 128 : (i + 1) * 128, :], in_=alpha_t
            ))
            pipe[i] = (expT_t, recip)
        if i >= lag:
            expT_p, recip_p = pipe.pop(i - lag)
            po = ps2.tile([128, d], F32)
            for sc in range(n_sc):
                nc.tensor.matmul(
                    po,
                    lhsT=expT_p[:, sc, :],
                    rhs=x_nat[:, sc, :],
                    start=(sc == 0),
                    stop=(sc == n_sc - 1),
                )
            out_t = outp.tile([128, d], F32)
            nc.scalar.mul(out_t, po, recip_p)
            pending_copies.append(nc.gpsimd.dma_start(
                out=out_d[(i - lag) * 128 : (i - lag + 1) * 128, :], in_=out_t
            ))
    return pending_copies


def kernel(x: np.ndarray, U: np.ndarray):
    from concourse.bass_utils import run_bass_kernel_spmd

    xb = np.asarray(x).astype(ml_dtypes.bfloat16)
    up = np.zeros((LP, D), dtype=ml_dtypes.bfloat16)
    up[:L] = np.asarray(U).astype(ml_dtypes.bfloat16)

    nc = build_nc()
    in_maps = [{"x": np.ascontiguousarray(xb[i]), "u": up} for i in range(B)]
    res = run_bass_kernel_spmd(nc, in_maps, list(range(B))).results

    out = np.empty((B, L, D), dtype=np.float32)
    alpha = np.empty((B, L, S), dtype=np.float32)
    for i in range(B):
        out[i] = res[i]["out"][:L]
        alpha[i] = res[i]["alpha"][:L]
    return out, alpha


# revision 28
# speedup vs baseline: 3.4767x; 1.7117x over previous
"""Label-wise attention (CAML-style) on 8 TRN2 NeuronCores.

scores = U @ x^T        [B, L, S]
alpha  = softmax(scores, axis=S)
out    = alpha @ x      [B, L, D]
returns (out, alpha)

Sharding: batch B=8 across the 8 cores (1 batch each, full U per core).
No collectives needed.

Per-core kernel (bf16 matmuls, fp32 softmax/outputs):
  - x_b resident in SBUF in natural [s,d] layout (for alpha@x) and
    transposed [d,s] layout (for U@x^T), both bf16; transposed copies are
    made with batched DMA-xbar transposes (one instruction produces all
    128-column chunks: out[p, c, j] = in[j, c*128 + p]).
  - U streamed per 128-label tile, transposed on load the same way.
  - Per l-tile: PE matmul -> scores PSUM; ScalarE Exp (+accum_out row
    sums) -> bf16 exp tile; DVE scales to fp32 alpha; SP issues ALL
    SDMA work in program order (ut xpose, one batched exp xpose per
    scores quarter, then the alpha/out HBM copies) so the xbar-mode
    transpose<->copy serialization reduces to SP's natural sequencing;
    PE matmul2 accumulates over 32 s-chunks, lagging two tiles so it
    never stalls on the exp/transpose chain; ScalarE copy-scales by
    1/rowsum -> fp32 out.
"""

import sys
from contextlib import ExitStack

for _p in ("/opt/trn_rl_repo", "/opt/pypackages"):
    if _p not in sys.path:
        sys.path.insert(0, _p)

import numpy as np
import ml_dtypes

import concourse.bass as bass
import concourse.mybir as mybir
import concourse.tile as tile
from concourse import bacc

B, S, D, L = 8, 4096, 512, 8921
LP = 8960  # L padded to 70 * 128
BF16 = mybir.dt.bfloat16
F32 = mybir.dt.float32


def build_nc(s=S, n_lt=LP // 128, repeat=1):
    d = D
    n_dc = d // 128   # d-chunks of 128 (contraction for matmul1)
    n_sc = s // 128   # s-chunks of 128 (contraction for matmul2)
    qw = 1024 if s % 1024 == 0 else 512  # matmul1 PSUM chunk width
    n_q = s // qw
    lp = n_lt * 128

    nc = bacc.Bacc()
    x_d = nc.declare_dram_parameter("x", [s, d], BF16, isOutput=False)
    u_d = nc.declare_dram_parameter("u", [lp, d], BF16, isOutput=False)
    out_d = nc.declare_dram_parameter("out", [lp, d], F32, isOutput=True)
    alpha_d = nc.declare_dram_parameter("alpha", [lp, s], F32, isOutput=True)

    with tile.TileContext(nc) as tc, ExitStack() as ctx:
        const = ctx.enter_context(tc.tile_pool(name="const", bufs=1))
        utp = ctx.enter_context(tc.tile_pool(name="ut", bufs=3))
        expp = ctx.enter_context(tc.tile_pool(name="exp", bufs=3))
        expTp = ctx.enter_context(tc.tile_pool(name="expT", bufs=4))
        alphap = ctx.enter_context(tc.tile_pool(name="alpha", bufs=3))
        outp = ctx.enter_context(tc.tile_pool(name="out", bufs=3))
        statp = ctx.enter_context(tc.tile_pool(name="stat", bufs=4))
        ps1 = ctx.enter_context(tc.tile_pool(name="ps1", bufs=3, space="PSUM"))
        ps2 = ctx.enter_context(tc.tile_pool(name="ps2", bufs=2, space="PSUM"))

        # x transposed (one batched xpose): xT[p, dc, j] = x[j, dc*128 + p]
        xT = const.tile([128, n_dc, s], BF16)
        nc.sync.dma_start(out=xT, in_=x_d[:, :], transpose=True)
        # x natural: x_nat[p, c, :] = x[c*128 + p, :]
        x_nat = const.tile([128, n_sc, d], BF16)
        x_nat_cp = nc.gpsimd.dma_start(
            out=x_nat, in_=x_d[:].rearrange("(c p) d -> p c d", p=128)
        )

        pools = (utp, expp, expTp, alphap, outp, statp, ps1, ps2)
        carry = [x_nat_cp]
        for _ in range(repeat):
            carry = _body(nc, n_lt, n_dc, n_sc, n_q, qw, s, d,
                          u_d, out_d, alpha_d, x_nat, xT, pools,
                          startup_copies=carry)
    nc.compile()
    return nc


def _body(nc, n_lt, n_dc, n_sc, n_q, qw, s, d,
          u_d, out_d, alpha_d, x_nat, xT, pools, lag=2,
          startup_copies=None):
    import os
    pin = os.environ.get("KPIN", "0") == "1"
    from concourse.tile import add_dep_helper
    (utp, expp, expTp, alphap, outp, statp, ps1, ps2) = pools
    pipe = {}
    # SWDGE copies emitted since the last pin point.  Each is ordered after
    # the NEXT tile's final xbar transpose (sync=False scheduler hint) so the
    # xbar-mode serialization never leaves a late copy wedged between a
    # tile's transposes, which would stall them (and then the PE).
    pending_copies = list(startup_copies or [])
    for i in range(n_lt + lag):
        if i < n_lt:
            # UT[p, dc, j] = u[i*128 + j, dc*128 + p]  (one batched xpose)
            ut = utp.tile([128, n_dc, 128], BF16)
            utx = nc.sync.dma_start(
                out=ut, in_=u_d[i * 128 : (i + 1) * 128, :], transpose=True
            )
            if pin:
                for cp in pending_copies:
                    add_dep_helper(cp.ins, utx.ins, sync=True,
                                   reason="copy after next tile's ut xpose")
            pending_copies = []
            exp_t = expp.tile([128, s], BF16)
            expT_t = expTp.tile([128, n_sc, 128], BF16)
            psums = statp.tile([128, n_q], F32, tag="psums")
            nq_c = qw // 128  # xpose chunks per quarter
            for q in range(n_q):
                ps = ps1.tile([128, qw], F32)
                for h in range(qw // 512):
                    for dc in range(n_dc):
                        nc.tensor.matmul(
                            ps[:, h * 512 : (h + 1) * 512],
                            lhsT=ut[:, dc, :],
                            rhs=xT[:, dc, q * qw + h * 512 : q * qw + (h + 1) * 512],
                            start=(dc == 0),
                            stop=(dc == n_dc - 1),
                        )
                nc.scalar.activation(
                    out=exp_t[:, q * qw : (q + 1) * qw],
                    in_=ps[:],
                    func=mybir.ActivationFunctionType.Exp,
                    accum_out=psums[:, q : q + 1],
                )
                # expT[p, qc, j] = exp_t[j, qc*128 + p] for this quarter
                xp = nc.sync.dma_start(
                    out=expT_t[:, q * nq_c : (q + 1) * nq_c, :],
                    in_=exp_t[:, q * qw : (q + 1) * qw],
                    transpose=True,
                )
            ssum = statp.tile([128, 1], F32, tag="ssum")
            recip = statp.tile([128, 1], F32, tag="recip")
            nc.vector.reduce_sum(out=ssum, in_=psums, axis=mybir.AxisListType.X)
            nc.vector.reciprocal(recip, ssum)
            alpha_t = alphap.tile([128, s], F32)
            nc.vector.tensor_scalar_mul(alpha_t, exp_t, recip)
            pending_copies.append(nc.sync.dma_start(
                out=alpha_d[i * 128 : (i + 1) * 128, :], in_=alpha_t
            ))
            pipe[i] = (expT_t, recip)
        if i >= lag:
            expT_p, recip_p = pipe.pop(i - lag)
            po = ps2.tile([128, d], F32)
            for sc in range(n_sc):
                nc.tensor.matmul(
                    po,
                    lhsT=expT_p[:, sc, :],
                    rhs=x_nat[:, sc, :],
                    start=(sc == 0),
                    stop=(sc == n_sc - 1),
                )
            out_t = outp.tile([128, d], F32)
            nc.scalar.mul(out_t, po, recip_p)
            pending_copies.append(nc.sync.dma_start(
                out=out_d[(i - lag) * 128 : (i - lag + 1) * 128, :], in_=out_t
            ))
    return pending_copies


def kernel(x: np.ndarray, U: np.ndarray):
    from concourse.bass_utils import run_bass_kernel_spmd

    xb = np.asarray(x).astype(ml_dtypes.bfloat16)
    up = np.zeros((LP, D), dtype=ml_dtypes.bfloat16)
    up[:L] = np.asarray(U).astype(ml_dtypes.bfloat16)

    nc = build_nc()
    in_maps = [{"x": np.ascontiguousarray(xb[i]), "u": up} for i in range(B)]
    res = run_bass_kernel_spmd(nc, in_maps, list(range(B))).results

    out = np.empty((B, L, D), dtype=np.float32)
    alpha = np.empty((B, L, S), dtype=np.float32)
    for i in range(B):
        out[i] = res[i]["out"][:L]
        alpha[i] = res[i]["alpha"][:L]
    return out, alpha
